# revision 18
# baseline (speedup 1.0000x reference)
"""GQA attention kernel for Trainium2 (8 NeuronCores, Bass/Tile) — v4.

Problem: B=2, S=2048, D=3072, 24 Q heads / 8 KV heads, HD=128, RoPE,
causal mask, softmax, output projection.

Sharding: tensor-parallel over heads. Core h owns KV head h and Q heads
{3h, 3h+1, 3h+2} for BOTH batch elements. Each core produces a partial
y^T = wo_slice^T.T @ attn_out_heads^T of shape (B, D, S) in fp16; the
host sums the 8 partials in fp32 and transposes back.

Performance structure (see the trace notes in the repo history):
  - The in-order PE queue must never wait: scores are emitted 4 key
    tiles ahead of their exp->AV consumers through a 5-bank PSUM ring,
    and every cross-engine chain (RoPE, softmax denominators, output
    copies) is emitted so its latency hides under independent PE work.
    PE stalls also halve the DVFS clock (216ns/512-row matmul busy vs
    584ns after a stall), so stall avoidance pays twice.
  - DMA descriptor ISSUE is the scarce resource, not DMA bandwidth:
    each dma_start costs ~600ns on its issuing engine's in-order
    sequencer, and a descriptor that waits on a semaphore blocks all
    later ones on that engine. Weight/xt loads are therefore batched
    2-contraction-tiles per descriptor (einops-split rearrange of the
    DRAM AP), split between the Sync and Scalar HWDGE sequencers, and
    the RoPE rotate-half SBUF->SBUF DMAs are issued from Scalar right
    after ACT's own staging copy (they'd block Sync's xt stream).
  - Causal diagonal 128-col narrowing (moving operands 512/384/256/128
    wide) + a single constant 128x128 triangular exp-mask tile on DVE.
  - fp16 output partials; f32r softmax-reciprocal broadcast matmul.
"""

import math
import os
import sys

import numpy as np

for _p in ("/opt/trn_rl_repo",):
    if os.path.isdir(_p) and _p not in sys.path:
        sys.path.insert(0, _p)

import concourse.bass as bass  # noqa: E402
import concourse.mybir as mybir  # noqa: E402
import concourse.tile as tile  # noqa: E402
from concourse import bacc  # noqa: E402
from concourse.bass_utils import run_bass_kernel_spmd  # noqa: E402

F32 = mybir.dt.float32
F32R = mybir.dt.float32r
F16 = mybir.dt.float16
AFT = mybir.ActivationFunctionType

N_CORES = 8
NSLOT = 4          # score-ring slots (PSUM banks)
LOOKAHEAD = 4      # score matmuls emitted ahead of their AV consumer

# Set by test harness to capture a profile on the next kernel() call.
TRACE = False
LAST_EXEC_NS = None
LAST_RESULTS = None


class Cfg:
    def __init__(self, B=2, S=2048, D=3072, QH=3, HD=128, SC=512):
        self.B, self.S, self.D, self.QH, self.HD, self.SC = B, S, D, QH, HD, SC
        self.CT = D // 128        # contraction tiles for projections
        self.KT = S // 128        # key tiles
        self.NSC = S // SC        # token chunks
        self.PB = SC // 128       # 128-blocks per token chunk
        self.SCALE = 1.0 / math.sqrt(HD)


def build_program(cfg):
    B, S, D, QH, HD, SC = cfg.B, cfg.S, cfg.D, cfg.QH, cfg.HD, cfg.SC
    CT, NSC, PB = cfg.CT, cfg.NSC, cfg.PB

    nc = bacc.Bacc("TRN2", target_bir_lowering=False, debug=False,
                   num_devices=N_CORES)

    xT = nc.declare_dram_parameter("xT", [B, D, S], F16, isOutput=False)
    cosT = nc.declare_dram_parameter("cosT", [HD, S], F32, isOutput=False)
    sinT = nc.declare_dram_parameter("sinT", [HD, S], F32, isOutput=False)
    wq = nc.declare_dram_parameter("wq", [D, QH * HD], F16, isOutput=False)
    wk = nc.declare_dram_parameter("wk", [D, HD], F16, isOutput=False)
    wv = nc.declare_dram_parameter("wv", [D, HD], F16, isOutput=False)
    wo = nc.declare_dram_parameter("wo", [QH * HD, D], F16, isOutput=False)
    tmask = nc.declare_dram_parameter("tmask", [128, 128], F16, isOutput=False)
    ident = nc.declare_dram_parameter("ident", [128, 128], F16, isOutput=False)
    onesc = nc.declare_dram_parameter("onesc", [128, 1], F16, isOutput=False)
    onesr = nc.declare_dram_parameter("onesr", [1, 128], F16,
                                       isOutput=False)
    yT = nc.declare_dram_parameter("yT", [B, D, S], F16, isOutput=True)

    xT_ap, yT_ap = xT.ap(), yT.ap()

    def split2(ap2d, ct0, n):
        """[n*128, C] DRAM rows ct0*128.. as a [128, n, C] transposed view."""
        return ap2d[ct0 * 128:(ct0 + n) * 128, :].rearrange(
            "(j p) c -> p j c", p=128)

    with tile.TileContext(nc) as tc:
        from contextlib import ExitStack
        with ExitStack() as top:
            const = top.enter_context(tc.tile_pool(name="const", bufs=1))
            stream = top.enter_context(tc.tile_pool(name="stream", bufs=1))
            # proj scratch and attention SBUF live in distinct top-level
            # pools: per-batch scoping would overlap the attention e-tiles
            # with the RoPE scratch, making the first exp of each batch
            # wait for the last chunk's whole RoPE pipeline to drain.
            sp = top.enter_context(tc.tile_pool(name="proj_scratch", bufs=1))
            asb = top.enter_context(tc.tile_pool(name="attn_sb", bufs=1))

            wq_sb = const.tile([128, CT, QH * HD], F16, name="wq_sb")
            wk_sb = const.tile([128, CT, HD], F16, name="wk_sb")
            wv_sb = const.tile([128, CT, HD], F16, name="wv_sb")
            wo_sb = const.tile([128, QH, D], F16, name="wo_sb")
            t_mask = const.tile([128, 128], F16, name="t_mask")
            id_sb = const.tile([128, 128], F16, name="id_sb")
            ones_col = const.tile([128, 1], F16, name="ones_col")
            ones_row = const.tile([1, 128], F16, name="ones_row")
            cos_sb = const.tile([128, NSC, SC], F32, name="cos_sb")
            sin_sb = const.tile([128, NSC, SC], F32, name="sin_sb")

            wo_loaded = False

            def new_xt2(b, ctp, sc, split=False):
                """One descriptor loading cts (2*ctp, 2*ctp+1) of chunk sc.
                split=True uses two half-descriptors so the first ct's
                matmuls can start as soon as its half lands."""
                cs = slice(sc * SC, (sc + 1) * SC)
                xt2 = stream.tile([128, 2, SC], F16, tag="x", bufs=20,
                                  name="xt2")
                for j2 in range(2):
                    ct = 2 * ctp + j2
                    # chunks >= 1: odd halves issue from Scalar — the Sync
                    # sequencer alone can't sustain 24 descriptors/chunk
                    # of issue cost once queued behind earlier chunks.
                    eng = nc.scalar if (sc > 0 and j2 == 1) else nc.sync
                    eng.dma_start(
                        xt2[:, j2, :],
                        xT_ap[b, ct * 128:(ct + 1) * 128, cs])
                return xt2

            # next batch's first xt tiles, prefetched during attention.
            xt_prefetch = {}

            # Startup order matters: the first chunk's xt tiles and first
            # cts' weights go out FIRST (interleaved, split across the Sync
            # and Scalar HWDGE sequencers at ~600ns/descriptor issue cost);
            # bulk weights, cos/sin and small constants follow. Everything
            # the first matmuls need lands within ~10us instead of sitting
            # behind ~40 descriptors of constants.
            for ct in range(8):
                rsl = slice(ct * 128, (ct + 1) * 128)
                nc.sync.dma_start(wk_sb[:, ct, :], wk.ap()[rsl, :])
                nc.scalar.dma_start(wq_sb[:, ct, :], wq.ap()[rsl, :])
                nc.scalar.dma_start(wv_sb[:, ct, :], wv.ap()[rsl, :])
                if ct % 2 == 0:
                    xt_prefetch[(0, ct // 2, 0)] = new_xt2(0, ct // 2, 0,
                                                           split=True)

            for ct in range(8, CT, 2):
                nc.sync.dma_start(wk_sb[:, ct:ct + 2, :],
                                  split2(wk.ap(), ct, 2))
                nc.scalar.dma_start(wv_sb[:, ct:ct + 2, :],
                                    split2(wv.ap(), ct, 2))
                nc.scalar.dma_start(wq_sb[:, ct:ct + 2, :],
                                    split2(wq.ap(), ct, 2))
                xt_prefetch[(0, ct // 2, 0)] = new_xt2(0, ct // 2, 0,
                                                       split=True)
            for s_ in range(NSC):
                cs = slice(s_ * SC, (s_ + 1) * SC)
                nc.sync.dma_start(cos_sb[:, s_, :], cosT.ap()[:, cs])
                nc.scalar.dma_start(sin_sb[:, s_, :], sinT.ap()[:, cs])
            nc.sync.dma_start(t_mask[:], tmask.ap())
            nc.sync.dma_start(id_sb[:], ident.ap())
            nc.sync.dma_start(ones_col[:], onesc.ap())
            nc.sync.dma_start(ones_row[:], onesr.ap())

            for b in range(B):
                with ExitStack() as bctx:
                    bpool = bctx.enter_context(
                        tc.tile_pool(name=f"b{b}_persist", bufs=1))
                    K_cks = [bpool.tile([128, SC], F16, name=f"K_sb{b}_{s_}")
                             for s_ in range(NSC)]
                    V_cks = [bpool.tile([128, PB, 128], F16,
                                        name=f"V_sb{b}_{s_}")
                             for s_ in range(NSC)]
                    Q_cks = [[bpool.tile([128, SC], F16,
                                         name=f"Q_sb{b}_{i}_{s_}")
                              for s_ in range(NSC)] for i in range(QH)]

                    # ---------------- QKV projection + RoPE ----------------
                    with ExitStack() as pctx:
                        pps = pctx.enter_context(
                            tc.tile_pool(name=f"b{b}_qkv_ps", bufs=1,
                                         space="PSUM"))

                        for sc in range(NSC):
                            accK = pps.tile([128, SC], F32, tag="accK",
                                            bufs=2, name="accK")
                            accQ = [pps.tile([128, SC], F32, tag=f"accQ{j}",
                                             bufs=1, name=f"accQ{j}")
                                    for j in range(QH)]
                            accV = pps.tile([128, SC], F32, tag="accV",
                                            bufs=1, name="accV")
                            for ctp in range(CT // 2):
                                xt2 = xt_prefetch.pop((b, ctp, sc), None)
                                if xt2 is None:
                                    xt2 = new_xt2(b, ctp, sc)
                                for j2 in range(2):
                                    ct = 2 * ctp + j2
                                    xr = xt2[:, j2, :]
                                    st = (ct == 0)
                                    sp_ = (ct == CT - 1)
                                    nc.tensor.matmul(
                                        accK[:], wk_sb[:, ct, :], xr,
                                        start=st, stop=sp_)
                                    for j in range(QH):
                                        nc.tensor.matmul(
                                            accQ[j][:],
                                            wq_sb[:, ct,
                                                  j * HD:(j + 1) * HD],
                                            xr, start=st, stop=sp_)
                                    nc.tensor.matmul(
                                        accV[:], wv_sb[:, ct, :], xr,
                                        start=st, stop=sp_)

                            if b == 0 and sc == 0 and not wo_loaded:
                                wo_loaded = True
                                for hh in range(QH):
                                    for q4 in range(4):
                                        csl = slice(q4 * (D // 4),
                                                    (q4 + 1) * (D // 4))
                                        nc.scalar.dma_start(
                                            wo_sb[:, hh, csl],
                                            wo.ap()[hh * 128:(hh + 1) * 128,
                                                    csl])

                            # V first: its transpose chain (PE->DVE
                            # ping-pong) must not queue behind the RoPE DVE
                            # muls, or the last chunk's V blocks attention.
                            vstage = sp.tile([128, SC], F16, tag="vst",
                                             bufs=2, name="vstage")
                            nc.scalar.copy(vstage[:], accV[:])
                            for j in range(PB):
                                v_ps = pps.tile([128, 128], F16, tag="vtr",
                                                bufs=2, name="v_ps")
                                nc.tensor.transpose(
                                    v_ps[:], vstage[:, j * 128:(j + 1) * 128],
                                    id_sb[:])
                                nc.vector.tensor_copy(V_cks[sc][:, j, :],
                                                      v_ps[:])

                            # RoPE on K then the QH q-heads. The PSUM acc is
                            # freed by the single ACT copy; the rotate DMAs
                            # issue from ACT's own HWDGE queue so the Sync
                            # sequencer never blocks on them.
                            # All four PSUM-freeing ACT copies run
                            # before any rot DMA issue: the attention (or
                            # next chunk's) matmuls reuse these banks and
                            # would otherwise wait for ACT to work through
                            # interleaved copy+rot backlog.
                            rope = [(accK, K_cks[sc])] + [
                                (accQ[j], Q_cks[j][sc]) for j in range(QH)]
                            t_sbs = []
                            for di, (acc, dst) in enumerate(rope):
                                t_sb = sp.tile([128, SC], F32, tag="tsb",
                                               bufs=4, name="t_sb")
                                # split across ACT+DVE: halves the serial
                                # latency until the PSUM banks recycle.
                                if di % 2 == 0:
                                    nc.scalar.copy(t_sb[:], acc[:])
                                else:
                                    nc.vector.tensor_copy(t_sb[:], acc[:])
                                t_sbs.append(t_sb)
                            # the last chunk's rotate DMAs go via Sync (idle
                            # once the xt stream is done) so the attention
                            # exps right behind them on the ACT queue are
                            # not delayed ~5us at the phase boundary.
                            rot_eng = nc.sync if sc == NSC - 1 else nc.scalar
                            for (acc, dst), t_sb in zip(rope, t_sbs):
                                rot = sp.tile([128, SC], F32, tag="rot",
                                              bufs=2, name="rot")
                                # rotate-half via partition-shifted DMA;
                                # sign of the first half folded into sinT.
                                rot_eng.dma_start(rot[0:64, :],
                                                  t_sb[64:128, :])
                                rot_eng.dma_start(rot[64:128, :],
                                                  t_sb[0:64, :])
                                tmp1 = sp.tile([128, SC], F32, tag="tmp1",
                                               bufs=2, name="tmp1")
                                nc.vector.tensor_mul(tmp1[:], t_sb[:],
                                                     cos_sb[:, sc, :])
                                tmp2 = sp.tile([128, SC], F32, tag="tmp2",
                                               bufs=2, name="tmp2")
                                nc.vector.tensor_mul(tmp2[:], rot[:],
                                                     sin_sb[:, sc, :])
                                nc.vector.tensor_add(dst[:], tmp1[:],
                                                     tmp2[:])

                    if b + 1 < B:
                        # queue the next batch's first xt loads on Sync now;
                        # they fire during this batch's attention phase.
                        for ctp in range(6):
                            xt_prefetch[(b + 1, ctp, 0)] = new_xt2(
                                b + 1, ctp, 0)

                    # ------------- attention + out-projection -------------
                    with ExitStack() as actx:
                        aps = actx.enter_context(
                            tc.tile_pool(name=f"b{b}_attn_ps", bufs=1,
                                         space="PSUM"))

                        # score ring: NSLOT separate single-bank PSUM
                        # tiles. One multi-bank tile serializes at tile
                        # granularity (every scores matmul waits the latest
                        # exp read of the tile); separate tiles give precise
                        # slot-level WAR deps.
                        s_tiles = [aps.tile([128, SC], F32,
                                            name=f"s_ring{j}")
                                   for j in range(NSLOT)]
                        r_ps = aps.tile([1, SC], F32, name="r_ps")

                        for qc in range(NSC):
                            qs = slice(qc * SC, (qc + 1) * SC)
                            # (kt, width, query-col offset); diagonal tiles
                            # narrowed to their causally-valid column range.
                            kts = [(kt, SC, 0) for kt in range(PB * qc)]
                            kts += [(PB * qc + j, SC - 128 * j, 128 * j)
                                    for j in range(PB)]
                            L = len(kts)

                            def emit_sc(hh, i, kts=kts):
                                kt, W, o = kts[i]
                                s_ps = s_tiles[i % NSLOT]
                                kb, kj = divmod(kt, PB)
                                nc.tensor.matmul(
                                    s_ps[:, :W],
                                    K_cks[kb][:, kj * 128:(kj + 1) * 128],
                                    Q_cks[hh][qc][:, o:o + W],
                                    start=True, stop=True)
                                e = asb.tile([128, SC], F16, tag="e",
                                             bufs=14, name="e")
                                nc.scalar.activation(
                                    e[:, :W], s_ps[:, :W],
                                    AFT.Exp, scale=cfg.SCALE)
                                if kt >= PB * qc:
                                    nc.vector.tensor_mul(
                                        e[:, :128], e[:, :128], t_mask[:])
                                return e

                            def emit_av(i, e, av, kts=kts, L=L):
                                kt, W, o = kts[i]
                                kb, kj = divmod(kt, PB)
                                st, sp_ = (i == 0), (i == L - 1)
                                nc.tensor.matmul(
                                    av[:, o:o + W], V_cks[kb][:, kj, :],
                                    e[:, :W], start=st, stop=sp_)
                                nc.tensor.matmul(
                                    r_ps[:1, o:o + W], ones_col[:],
                                    e[:, :W], start=st, stop=sp_)

                            ohs = []
                            pend_fin = None
                            for hh in range(QH):
                                av = aps.tile([128, SC], F32, tag="avy",
                                              bufs=3, name="av")
                                invb = aps.tile([128, SC], F32, tag="avy",
                                                bufs=3, name="invb")
                                es = [emit_sc(hh, i)
                                      for i in range(min(LOOKAHEAD, L))]
                                if pend_fin is not None:
                                    pend_fin()
                                    pend_fin = None
                                for i in range(L):
                                    emit_av(i, es[i], av)
                                    if i + LOOKAHEAD < L:
                                        es.append(
                                            emit_sc(hh, i + LOOKAHEAD))

                                def fin(av=av, invb=invb):
                                    inv_sb = asb.tile([1, SC], F32,
                                                      tag="inv", bufs=2,
                                                      name="inv_sb")
                                    # r is a sum of positive exps -> the
                                    # ~18-bit fast approx is plenty.
                                    nc.vector.reciprocal_approx_fast(
                                        inv_sb[:], r_ps[:])
                                    inv_r = asb.tile([1, SC], F16,
                                                     tag="invr", bufs=2,
                                                     name="inv_r")
                                    nc.vector.tensor_copy(inv_r[:],
                                                          inv_sb[:])
                                    nc.tensor.matmul(invb[:], ones_row[:],
                                                     inv_r[:], start=True,
                                                     stop=True)
                                    invb_sb = asb.tile([128, SC], F32,
                                                       tag="invb_sb",
                                                       bufs=2,
                                                       name="invb_sb")
                                    nc.scalar.copy(invb_sb[:], invb[:])
                                    oh = asb.tile([128, SC], F16, tag="oh",
                                                  bufs=QH + 1, name="oh")
                                    nc.vector.tensor_mul(oh[:], av[:],
                                                         invb_sb[:])
                                    ohs.append(oh)
                                pend_fin = fin
                            # partial first out-proj tile (heads 0..QH-2)
                            # before the last head's fin: covers the PE
                            # while the softmax-denominator chain runs.
                            y_head = []
                            y_ps0 = aps.tile([128, SC], F32, tag="avy",
                                             bufs=3, name="y_ps")
                            y_head.append(y_ps0)
                            for hh in range(QH - 1):
                                nc.tensor.matmul(
                                    y_ps0[:], wo_sb[:, hh, 0:128],
                                    ohs[hh][:], start=(hh == 0), stop=False)
                            pend_fin()

                            for mt in range(CT):
                                if mt < 1:
                                    y_ps = y_head[mt]
                                    nc.tensor.matmul(
                                        y_ps[:],
                                        wo_sb[:, QH - 1,
                                              mt * 128:(mt + 1) * 128],
                                        ohs[QH - 1][:],
                                        start=False, stop=True)
                                else:
                                    y_ps = aps.tile([128, SC], F32,
                                                    tag="avy", bufs=3,
                                                    name="y_ps")
                                    for hh in range(QH):
                                        nc.tensor.matmul(
                                            y_ps[:],
                                            wo_sb[:, hh,
                                                  mt * 128:(mt + 1) * 128],
                                            ohs[hh][:],
                                            start=(hh == 0),
                                            stop=(hh == QH - 1))
                                y_sb = asb.tile([128, SC], F16, tag="ysb",
                                                bufs=6, name="y_sb")
                                # split PSUM->SBUF copies evenly; either
                                # engine alone is slower than the PE's
                                # 648ns/tile production rate.
                                if mt % 2 == 0:
                                    nc.vector.tensor_copy(y_sb[:], y_ps[:])
                                else:
                                    nc.scalar.copy(y_sb[:], y_ps[:])
                                nc.sync.dma_start(
                                    yT_ap[b, mt * 128:(mt + 1) * 128, qs],
                                    y_sb[:])

    nc.compile()
    return nc


def make_inputs(cfg, x, freqs_cos, freqs_sin, mask, wq, wk, wv, wo):
    """Host-side preprocessing -> per-core input maps."""
    B, S, QH, HD = cfg.B, cfg.S, cfg.QH, cfg.HD
    f32, f16 = np.float32, np.float16
    x = np.asarray(x, f32)
    xT = np.ascontiguousarray(np.transpose(x, (0, 2, 1)).astype(f16))
    cosT = np.ascontiguousarray(
        np.concatenate([freqs_cos, freqs_cos], axis=1).T.astype(f32))
    sinT = np.concatenate([freqs_sin, freqs_sin], axis=1).T.astype(f32).copy()
    sinT[:HD // 2] *= -1.0  # sign of rotate-half folded in
    sinT = np.ascontiguousarray(sinT)

    # The kernel hardcodes the causal block structure; verify the mask
    # matches and extract the 128x128 triangular multiplicative tile
    # (same pattern for every diagonal 128-block).
    m2 = np.asarray(mask, f32)[0, 0]
    assert np.array_equal(
        m2 != 0, ~np.tril(np.ones((S, S), dtype=bool))), "non-causal mask"
    tmask = np.ascontiguousarray(
        np.exp(m2[:128, :128]).T.astype(f16))  # [k, q]
    identity = np.ascontiguousarray(np.eye(128, dtype=f16))

    wqT = np.asarray(wq, f32).T.astype(f16)
    wkT = np.asarray(wk, f32).T.astype(f16)
    wvT = np.asarray(wv, f32).T.astype(f16)
    woT = np.asarray(wo, f32).T.astype(f16)

    in_maps = []
    for h in range(N_CORES):
        qsl = slice(h * QH * HD, (h + 1) * QH * HD)
        ksl = slice(h * HD, (h + 1) * HD)
        in_maps.append({
            "xT": xT,
            "cosT": cosT,
            "sinT": sinT,
            "wq": np.ascontiguousarray(wqT[:, qsl]),
            "wk": np.ascontiguousarray(wkT[:, ksl]),
            "wv": np.ascontiguousarray(wvT[:, ksl]),
            "wo": np.ascontiguousarray(woT[qsl, :]),
            "tmask": tmask,
            "ident": identity,
            "onesc": np.ones((128, 1), f16),
            "onesr": np.ones((1, 128), f16),
        })
    return in_maps


_CACHE = {}


def kernel(x, freqs_cos, freqs_sin, mask, wq, wk, wv, wo):
    global LAST_EXEC_NS, LAST_RESULTS
    cfg = Cfg()
    assert tuple(x.shape) == (cfg.B, cfg.S, cfg.D), x.shape

    in_maps = make_inputs(cfg, x, freqs_cos, freqs_sin, mask, wq, wk, wv, wo)

    if "v4" not in _CACHE:
        _CACHE["v4"] = build_program(cfg)
    nc = _CACHE["v4"]

    kwargs = {}
    if TRACE:
        kwargs = dict(trace=True, trace_cores=[0])
    res = run_bass_kernel_spmd(nc, in_maps, list(range(N_CORES)), **kwargs)
    LAST_EXEC_NS = res.exec_time_ns
    LAST_RESULTS = res

    acc = np.zeros((cfg.B, cfg.D, cfg.S), np.float32)
    for i in range(N_CORES):
        acc += res.results[i]["yT"].astype(np.float32)
    y = np.ascontiguousarray(np.transpose(acc, (0, 2, 1)))
    return y
